# revision 8
# baseline (speedup 1.0000x reference)
"""Nearest-color-distance loss on 8 TRN2 NeuronCores.

loss = mean_i min_j ||x_i - p_j||_2,  x: (131072, 3), p: (128, 3).

Per core (16384 colors): d2(i,j) = ||p_j||^2 - 2 x_i.p_j + ||x_i||^2.
The (pn - 2xp) part is one 16x128 @ 16x512 matmul per group of 512
colors (4 chunks of 128 colors block-diagonally packed into K=16),
min-reduced over the palette axis on DVE, then + ||x||^2, sqrt and a
free-axis accumulate on ACT. Host does layout-only prep + final mean.
"""

import sys

sys.path.insert(0, "/opt/trn_rl_repo")

import numpy as np

import concourse.bass as bass
import concourse.tile as tile
from concourse import bacc, mybir
from concourse.alu_op_type import AluOpType
from concourse.bass_utils import run_bass_kernel_spmd

N_CORES = 8
N = 131072
NPC = N // N_CORES  # 16384 colors per core
M = 128  # palette size
G = 32  # groups of 512 colors per core
F32 = mybir.dt.float32
F32R = mybir.dt.float32r
AF = mybir.ActivationFunctionType

# matmul operand dtype: float32r = full-rate PE; flip to F32 if precision fails
MM_DT = F32R


def build_nc():
    nc = bacc.Bacc(
        "TRN2",
        target_bir_lowering=False,
        debug=False,
        enable_asserts=False,
        num_devices=N_CORES,
    )
    xt_d = nc.dram_tensor("xt", [8, 16, 512], F32, kind="ExternalInput").ap()
    xn_d = nc.dram_tensor("xn", [128, 384], F32, kind="ExternalInput").ap()
    pt_d = nc.dram_tensor("palT", [3, 128], F32, kind="ExternalInput").ap()
    out_d = nc.dram_tensor("acc", [128, 1], F32, kind="ExternalOutput").ap()

    with tile.TileContext(nc) as tc:
        with (
            tc.tile_pool(name="sb", bufs=1) as sb,
            tc.tile_pool(name="ps", bufs=4, space=bass.MemorySpace.PSUM) as ps,
            tc.tile_pool(name="ps1", bufs=1, space=bass.MemorySpace.PSUM) as ps1,
        ):
            xt = sb.tile([16, 4096], MM_DT)
            xn = sb.tile([128, 384], F32)
            palT = sb.tile([3, 128], F32)
            sqT = sb.tile([3, 128], F32)
            ones3 = sb.tile([3, 1], F32)
            p16 = sb.tile([16, 512], MM_DT)
            minv = sb.tile([128, 128], F32)
            sq3 = sb.tile([128, 384], F32)
            xn2t = sb.tile([128, 128], F32)
            tot = sb.tile([128, 128], F32)
            sqv = sb.tile([128, 128], F32)
            acc = sb.tile([128, 1], F32)

            # input DMAs (xt chunked so matmuls can start early)
            nc.sync.dma_start(palT[:], pt_d[:])
            for q in range(8):
                nc.sync.dma_start(xt[:, bass.ts(q, 512)], xt_d[q].bitcast(MM_DT))
            nc.sync.dma_start(xn[:], xn_d[:])

            # palette prep: p16[4c+ch, c*128+j] = -2*pal[j,ch]; row 4c+3 = |p_j|^2
            nc.scalar.activation(sqT[:], palT[:], AF.Square)
            nc.gpsimd.memset(ones3[:], 1.0)
            pn_ps = ps1.tile([1, 128], F32)
            nc.tensor.matmul(pn_ps[:], ones3[:], sqT[:], start=True, stop=True)
            # memset can't write f32r — zero p16 via DMA from an f32 scratch
            zs = sb.tile([16, 512], F32)
            nc.gpsimd.memset(zs[:], 0.0)
            nc.sync.dma_start(p16[:], zs[:].bitcast(MM_DT))
            m2 = sb.tile([3, 128], F32)
            pn_sb = sb.tile([1, 128], F32)
            nc.scalar.mul(m2[:], palT[:], -2.0)
            nc.scalar.copy(pn_sb[:], pn_ps[:])
            # compute engines can't start at partitions 4/8/12 — DMA instead
            for c in range(4):
                nc.sync.dma_start(
                    p16[4 * c : 4 * c + 3, bass.ts(c, 128)], m2[:].bitcast(MM_DT)
                )
                nc.sync.dma_start(
                    p16[4 * c + 3 : 4 * c + 4, bass.ts(c, 128)],
                    pn_sb[:].bitcast(MM_DT),
                )

            # main loop: 32 groups of 512 colors
            for g in range(G):
                d_ps = ps.tile([128, 512], F32)
                nc.tensor.matmul(
                    d_ps[:],
                    xt[:, bass.ts(g, 128)],
                    p16[:],
                    start=True,
                    stop=True,
                )
                nc.vector.tensor_reduce(
                    minv[:, 4 * g : 4 * g + 4],
                    d_ps[:].rearrange("p (c j) -> p c j", j=128),
                    axis=mybir.AxisListType.X,
                    op=AluOpType.min,
                )

            # ||x||^2 in the transposed-color layout, + min, sqrt, row-sum
            nc.scalar.activation(sq3[:], xn[:], AF.Square)
            nc.vector.tensor_reduce(
                xn2t[:],
                sq3[:].rearrange("p (m c) -> p m c", c=3),
                axis=mybir.AxisListType.X,
                op=AluOpType.add,
            )
            nc.vector.tensor_add(tot[:], minv[:], xn2t[:])
            nc.vector.tensor_scalar_max(tot[:], tot[:], 0.0)
            nc.scalar.activation(sqv[:], tot[:], AF.Sqrt, accum_out=acc[:])
            nc.sync.dma_start(out_d[:], acc[:])

    nc.compile()
    return nc


def prep_inputs(output_colors, target_palette):
    x = np.ascontiguousarray(output_colors, dtype=np.float32)
    palT = np.ascontiguousarray(target_palette.astype(np.float32).T)  # (3,128)
    in_maps = []
    for k in range(N_CORES):
        xs = x[k * NPC : (k + 1) * NPC]  # (16384, 3)
        xr = xs.reshape(G, 4, 128, 3)  # [g, c, i, ch]
        xt = np.ones((4, 4, G, 128), dtype=np.float32)  # [c, ch4, g, i]
        xt[:, 0:3, :, :] = xr.transpose(1, 3, 0, 2)
        # rows 4c+ch over cols g*128+i, chunked to 8 contiguous [16,512] blocks
        xt = xt.reshape(16, 4096).reshape(16, 8, 512).transpose(1, 0, 2)
        xn = xs.reshape(128, 128, 3).transpose(1, 0, 2).reshape(128, 384)
        in_maps.append(
            {
                "xt": np.ascontiguousarray(xt),
                "xn": np.ascontiguousarray(xn),
                "palT": palT,
            }
        )
    return in_maps


_NC_CACHE = {}


def get_nc():
    if "nc" not in _NC_CACHE:
        _NC_CACHE["nc"] = build_nc()
    return _NC_CACHE["nc"]


def kernel(output_colors=None, target_palette=None, _trace=False, **_):
    nc = get_nc()
    in_maps = prep_inputs(output_colors, target_palette)
    res = run_bass_kernel_spmd(
        nc, in_maps, core_ids=list(range(N_CORES)), trace=_trace
    )
    total = np.float64(0.0)
    for r in res.results:
        total += np.sum(r["acc"].astype(np.float64))
    out = np.array(total / N, dtype=np.float32)
    if _trace:
        kernel._last_results = res
    return out


if __name__ == "__main__":
    rng = np.random.default_rng(0)
    oc = rng.random((N, 3), dtype=np.float32)
    tp = rng.random((M, 3), dtype=np.float32)
    got = kernel(output_colors=oc, target_palette=tp)
    d = oc[:, None, :] - tp[None, :, :]
    want = np.sqrt((d * d).sum(-1)).min(1).mean(dtype=np.float64)
    print("got", got, "want", want, "rel", abs(got - want) / abs(want))


# revision 9
# speedup vs baseline: 1.2415x; 1.2415x over previous
"""Nearest-color-distance loss on 8 TRN2 NeuronCores.

loss = mean_i min_j ||x_i - p_j||_2,  x: (131072, 3), p: (128, 3).

Per core (16384 colors): d2(i,j) = ||p_j||^2 - 2 x_i.p_j + ||x_i||^2
computed entirely inside one 20x128 @ 20x512 matmul per group of 512
colors: 4 chunks of 128 colors block-diagonally packed with 5 K-rows
each (x_ch, 1, ||x||^2 stationary vs -2p_ch, ||p||^2, 1 moving).
DVE min-reduces each PSUM bank over the palette axis, ACT does
sqrt + free-axis accumulate. Host does layout + centering prep
(subtract palette mean -- translation-invariant, shrinks f32r
cancellation error) and the final mean.
"""

import sys

sys.path.insert(0, "/opt/trn_rl_repo")

import numpy as np

import concourse.bass as bass
import concourse.tile as tile
from concourse import bacc, mybir
from concourse.alu_op_type import AluOpType
from concourse.bass_utils import run_bass_kernel_spmd

N_CORES = 8
N = 131072
NPC = N // N_CORES  # 16384 colors per core
M = 128  # palette size
G = 32  # groups of 512 colors per core
F32 = mybir.dt.float32
F32R = mybir.dt.float32r
AF = mybir.ActivationFunctionType

MM_DT = F32R  # full-rate PE dtype; flip to F32 if precision fails


def build_nc():
    nc = bacc.Bacc(
        "TRN2",
        target_bir_lowering=False,
        debug=False,
        enable_asserts=False,
        num_devices=N_CORES,
    )
    xt_d = nc.dram_tensor("xt", [20, 4096], F32, kind="ExternalInput").ap()
    p20_d = nc.dram_tensor("p20", [20, 512], F32, kind="ExternalInput").ap()
    out_d = nc.dram_tensor("acc", [128, 1], F32, kind="ExternalOutput").ap()

    with tile.TileContext(nc) as tc:
        with (
            tc.tile_pool(name="sb", bufs=1) as sb,
            tc.tile_pool(name="ps", bufs=6, space=bass.MemorySpace.PSUM) as ps,
        ):
            xt = sb.tile([20, 4096], MM_DT)
            p20 = sb.tile([20, 512], MM_DT)
            minv = sb.tile([128, 128], F32)
            tot = sb.tile([128, 128], F32)
            sqv = sb.tile([128, 128], F32)
            acc = sb.tile([128, 1], F32)
            junk = sb.tile([128, 1], F32)
            junk2 = sb.tile([128, 1], F32)

            nc.sync.dma_start(p20[:], p20_d.bitcast(MM_DT))
            nc.sync.dma_start(xt[:], xt_d.bitcast(MM_DT))

            # pre-warm the Sqrt activation table while DMAs run
            nc.gpsimd.memset(junk[:], 1.0)
            nc.scalar.activation(junk2[:], junk[:], AF.Sqrt)

            for g in range(G):
                d_ps = ps.tile([128, 512], F32)
                nc.tensor.matmul(
                    d_ps[:],
                    xt[:, bass.ts(g, 128)],
                    p20[:],
                    start=True,
                    stop=True,
                )
                nc.vector.tensor_reduce(
                    minv[:, 4 * g : 4 * g + 4],
                    d_ps[:].rearrange("p (c j) -> p c j", j=128),
                    axis=mybir.AxisListType.X,
                    op=AluOpType.min,
                )

            nc.vector.tensor_scalar_max(tot[:], minv[:], 0.0)
            nc.scalar.activation(sqv[:], tot[:], AF.Sqrt, accum_out=acc[:])
            nc.sync.dma_start(out_d[:], acc[:])

    nc.compile()
    return nc


def prep_inputs(output_colors, target_palette):
    pal = np.asarray(target_palette, dtype=np.float32)
    mu = pal.mean(axis=0)
    pp = pal - mu  # (128, 3) centered palette
    pn = (pp * pp).sum(axis=1)  # (128,)
    p20 = np.zeros((20, 512), dtype=np.float32)
    for c in range(4):
        p20[5 * c : 5 * c + 3, 128 * c : 128 * (c + 1)] = -2.0 * pp.T
        p20[5 * c + 3, 128 * c : 128 * (c + 1)] = pn
        p20[5 * c + 4, 128 * c : 128 * (c + 1)] = 1.0

    x = np.asarray(output_colors, dtype=np.float32) - mu
    in_maps = []
    for k in range(N_CORES):
        xs = x[k * NPC : (k + 1) * NPC]  # (16384, 3)
        xr = xs.reshape(G, 4, 128, 3)  # [g, c, i, ch]
        xn2 = (xs * xs).sum(axis=1).reshape(G, 4, 128)  # [g, c, i]
        xt = np.empty((4, 5, G, 128), dtype=np.float32)  # [c, row, g, i]
        xt[:, 0:3] = xr.transpose(1, 3, 0, 2)
        xt[:, 3] = 1.0
        xt[:, 4] = xn2.transpose(1, 0, 2)
        in_maps.append(
            {"xt": np.ascontiguousarray(xt.reshape(20, 4096)), "p20": p20}
        )
    return in_maps


_NC_CACHE = {}


def get_nc():
    if "nc" not in _NC_CACHE:
        _NC_CACHE["nc"] = build_nc()
    return _NC_CACHE["nc"]


def kernel(output_colors=None, target_palette=None, _trace=False, **_):
    nc = get_nc()
    in_maps = prep_inputs(output_colors, target_palette)
    res = run_bass_kernel_spmd(
        nc, in_maps, core_ids=list(range(N_CORES)), trace=_trace
    )
    total = np.float64(0.0)
    for r in res.results:
        total += np.sum(r["acc"].astype(np.float64))
    out = np.array(total / N, dtype=np.float32)
    if _trace:
        kernel._last_results = res
    return out


if __name__ == "__main__":
    rng = np.random.default_rng(0)
    oc = rng.random((N, 3), dtype=np.float32)
    tp = rng.random((M, 3), dtype=np.float32)
    got = kernel(output_colors=oc, target_palette=tp)
    d = oc[:, None, :] - tp[None, :, :]
    want = np.sqrt((d * d).sum(-1)).min(1).mean(dtype=np.float64)
    print("got", got, "want", want, "rel", abs(got - want) / abs(want))


# revision 17
# speedup vs baseline: 1.2841x; 1.0343x over previous
"""Nearest-color-distance loss on 8 TRN2 NeuronCores.

loss = mean_i min_j ||x_i - p_j||_2,  x: (131072, 3), p: (128, 3).

Per core (16384 colors): d2(i,j) = ||p_j||^2 - 2 x_i.p_j + ||x_i||^2
computed entirely inside the PE via 5-row packings (x_ch, 1, ||x||^2
against -2p_ch, ||p||^2, 1). Two layouts run interleaved so no single
reduction engine gates the loop:
 - 24 "bd" groups: 4 color-chunks block-diagonal (K=20) per matmul,
   colors on PSUM partitions; DVE min-reduces pairs of groups over
   the palette (free) axis.
 - 8 "sw" groups: palette stationary (K=5), colors moving; palette on
   PSUM partitions; ACT copies PSUM->SBUF and GpSimd min-reduces over
   the partition (C) axis.
Raw min-d2 go back to the host, which does sqrt/clamp/mean in f64,
plus layout + centering prep (centering shrinks f32r cancellation).
"""

import sys

sys.path.insert(0, "/opt/trn_rl_repo")

import numpy as np

import concourse.bass as bass
import concourse.bass_isa as bass_isa
import concourse.tile as tile
from concourse import bacc, mybir
from concourse.alu_op_type import AluOpType
from concourse.bass_utils import run_bass_kernel_spmd

N_CORES = 8
N = 131072
NPC = N // N_CORES  # 16384 colors per core
M = 128  # palette size
G = 32  # groups of 512 colors per core
BD = 24  # block-diagonal groups (12 PSUM pairs, DVE-consumed)
SW = 8  # swapped-layout groups (ACT+GpSimd-consumed)
NBD = BD * 512  # 12288 colors via bd path
F32 = mybir.dt.float32
F32R = mybir.dt.float32r
AF = mybir.ActivationFunctionType

MM_DT = F32R  # full-rate PE dtype; flip to F32 if precision fails


def build_nc():
    nc = bacc.Bacc(
        "TRN2",
        target_bir_lowering=False,
        debug=False,
        enable_asserts=False,
        num_devices=N_CORES,
    )
    xt_d = nc.dram_tensor("xt", [2, 20, 1536], F32, kind="ExternalInput").ap()
    p20_d = nc.dram_tensor("p20", [20, 512], F32, kind="ExternalInput").ap()
    aux_d = nc.dram_tensor("aux", [5, 4224], F32, kind="ExternalInput").ap()
    minv_d = nc.dram_tensor("minv", [128, 96], F32, kind="ExternalOutput").ap()
    minr_d = nc.dram_tensor("minr", [1, 4096], F32, kind="ExternalOutput").ap()

    with tile.TileContext(nc) as tc:
        with (
            tc.tile_pool(name="sb", bufs=1) as sb,
            tc.tile_pool(name="cp", bufs=2) as cpp,
            tc.tile_pool(name="pp", bufs=2, space=bass.MemorySpace.PSUM) as pp,
            tc.tile_pool(name="pw", bufs=3, space=bass.MemorySpace.PSUM) as pw,
        ):
            xt = sb.tile([20, 3072], MM_DT)
            p20 = sb.tile([20, 512], MM_DT)
            aux = sb.tile([5, 4224], MM_DT)
            minv = sb.tile([128, 96], F32)
            allres = sb.tile([128, 4096], F32)

            nc.sync.dma_start(xt[:, 0:1536], xt_d[0].bitcast(MM_DT))
            nc.sync.dma_start(aux[:], aux_d.bitcast(MM_DT))
            nc.scalar.dma_start(p20[:], p20_d.bitcast(MM_DT))
            nc.gpsimd.dma_start(xt[:, 1536:3072], xt_d[1].bitcast(MM_DT))
            pal5 = aux[:, 0:128]
            xs5 = aux[:, 128:4224]

            def bd_pair(p):
                d_ps = pp.tile([128, 1024], F32)
                for h in range(2):
                    nc.tensor.matmul(
                        d_ps[:, 512 * h : 512 * (h + 1)],
                        xt[:, bass.ts(2 * p + h, 128)],
                        p20[:],
                        start=True,
                        stop=True,
                    )
                nc.vector.tensor_reduce(
                    minv[:, 8 * p : 8 * p + 8],
                    d_ps[:].rearrange("p (c j) -> p c j", j=128),
                    axis=mybir.AxisListType.X,
                    op=AluOpType.min,
                )

            def sw_one(s):
                d_ps = pw.tile([128, 512], F32)
                nc.tensor.matmul(
                    d_ps[:],
                    pal5[:],
                    xs5[:, bass.ts(s, 512)],
                    start=True,
                    stop=True,
                )
                cp = cpp.tile([128, 512], F32)
                nc.scalar.mul(cp[:], d_ps[:], -1.0)
                nc.gpsimd.partition_all_reduce(
                    allres[:, bass.ts(s, 512)],
                    cp[:],
                    channels=128,
                    reduce_op=bass_isa.ReduceOp.max,
                )

            for r in range(4):
                for q in range(3):
                    bd_pair(3 * r + q)
                for q in range(2):
                    sw_one(2 * r + q)

            nc.sync.dma_start(minv_d[:], minv[:])
            nc.sync.dma_start(minr_d[:], allres[0:1, :])

    nc.compile()
    return nc


def prep_inputs(output_colors, target_palette):
    pal = np.asarray(target_palette, dtype=np.float32)
    mu = pal.mean(axis=0)
    pp = pal - mu  # (128, 3) centered palette
    pn = (pp * pp).sum(axis=1)  # (128,)
    p20 = np.zeros((20, 512), dtype=np.float32)
    for c in range(4):
        p20[5 * c : 5 * c + 3, 128 * c : 128 * (c + 1)] = -2.0 * pp.T
        p20[5 * c + 3, 128 * c : 128 * (c + 1)] = pn
        p20[5 * c + 4, 128 * c : 128 * (c + 1)] = 1.0

    x = np.asarray(output_colors, dtype=np.float32) - mu
    in_maps = []
    for k in range(N_CORES):
        xs = x[k * NPC : (k + 1) * NPC]  # (16384, 3)
        xn2 = (xs * xs).sum(axis=1)  # (16384,)

        xb = xs[:NBD].reshape(BD, 4, 128, 3)  # [g, c, i, ch]
        nb = xn2[:NBD].reshape(BD, 4, 128)
        xt = np.empty((4, 5, BD, 128), dtype=np.float32)  # [c, row, g, i]
        xt[:, 0:3] = xb.transpose(1, 3, 0, 2)
        xt[:, 3] = 1.0
        xt[:, 4] = nb.transpose(1, 0, 2)
        xt = xt.reshape(20, 2, 1536).transpose(1, 0, 2)  # 2 column-chunks

        aux = np.empty((5, 4224), dtype=np.float32)
        aux[0:3, 0:128] = -2.0 * pp.T
        aux[3, 0:128] = pn
        aux[4, 0:128] = 1.0
        aux[0:3, 128:] = xs[NBD:].T
        aux[3, 128:] = 1.0
        aux[4, 128:] = xn2[NBD:]

        in_maps.append(
            {
                "xt": np.ascontiguousarray(xt),
                "p20": p20,
                "aux": aux,
            }
        )
    return in_maps


_NC_CACHE = {}


def get_nc():
    if "nc" not in _NC_CACHE:
        _NC_CACHE["nc"] = build_nc()
    return _NC_CACHE["nc"]


def kernel(output_colors=None, target_palette=None, _trace=False, **_):
    nc = get_nc()
    in_maps = prep_inputs(output_colors, target_palette)
    res = run_bass_kernel_spmd(
        nc, in_maps, core_ids=list(range(N_CORES)), trace=_trace
    )
    total = np.float64(0.0)
    for r in res.results:
        d2b = np.maximum(r["minv"].astype(np.float64), 0.0)
        d2s = np.maximum(-r["minr"].astype(np.float64), 0.0)
        total += np.sqrt(d2b).sum() + np.sqrt(d2s).sum()
    out = np.array(total / N, dtype=np.float32)
    if _trace:
        kernel._last_results = res
    return out


if __name__ == "__main__":
    rng = np.random.default_rng(0)
    oc = rng.random((N, 3), dtype=np.float32)
    tp = rng.random((M, 3), dtype=np.float32)
    got = kernel(output_colors=oc, target_palette=tp)
    d = oc[:, None, :] - tp[None, :, :]
    want = np.sqrt((d * d).sum(-1)).min(1).mean(dtype=np.float64)
    print("got", got, "want", want, "rel", abs(got - want) / abs(want))


# revision 19
# speedup vs baseline: 1.4872x; 1.1582x over previous
"""Nearest-color-distance loss on 8 TRN2 NeuronCores.

loss = mean_i min_j ||x_i - p_j||_2,  x: (131072, 3), p: (128, 3).

Per core (16384 colors): d2(i,j) = ||p_j||^2 - 2 x_i.p_j + ||x_i||^2
computed entirely inside the PE via 5-row packings (x_ch, 1, ||x||^2
against -2p_ch, ||p||^2, 1). Two layouts run interleaved so no single
reduction engine gates the loop:
 - 25 "bd" groups: 4 color-chunks block-diagonal (K=20) per matmul,
   colors on PSUM partitions; DVE min-reduces pairs of groups over
   the palette (free) axis (12 pairs + 1 single).
 - 7 "sw" groups: palette stationary (K=5), colors moving; palette on
   PSUM partitions; ACT copies PSUM->SBUF and GpSimd min-reduces over
   the partition (C) axis.
sw groups are front-loaded in program order and their inputs land
first so the slow GpSimd chain starts ~10us in; the schedule ends on
the cheap single-group DVE MIN. Raw min-d2 go back to the host, which
does sqrt/clamp/mean in f64, plus layout + centering prep.
"""

import sys

sys.path.insert(0, "/opt/trn_rl_repo")

import numpy as np

import concourse.bass as bass
import concourse.bass_isa as bass_isa
import concourse.tile as tile
from concourse import bacc, mybir
from concourse.alu_op_type import AluOpType
from concourse.bass_utils import run_bass_kernel_spmd

N_CORES = 8
N = 131072
NPC = N // N_CORES  # 16384 colors per core
M = 128  # palette size
BD = 25  # block-diagonal groups of 512 colors (12 pairs + 1 single)
SW = 7  # swapped-layout groups (ACT+GpSimd-consumed)
NBD = BD * 512  # 12800 colors via bd path
NSW = NPC - NBD  # 3584 colors via sw path
GA = 13  # bd groups in xtA (pairs 0-5 + single), rest in xtB
F32 = mybir.dt.float32
F32R = mybir.dt.float32r
AF = mybir.ActivationFunctionType

MM_DT = F32R  # full-rate PE dtype; flip to F32 if precision fails


def build_nc():
    nc = bacc.Bacc(
        "TRN2",
        target_bir_lowering=False,
        debug=False,
        enable_asserts=False,
        num_devices=N_CORES,
    )
    xta_d = nc.dram_tensor("xta", [20, 128 * GA], F32, kind="ExternalInput").ap()
    xtb_d = nc.dram_tensor("xtb", [20, 128 * (BD - GA)], F32, kind="ExternalInput").ap()
    p20_d = nc.dram_tensor("p20", [20, 512], F32, kind="ExternalInput").ap()
    auxa_d = nc.dram_tensor("auxa", [5, 1152], F32, kind="ExternalInput").ap()
    xsb_d = nc.dram_tensor("xsb", [5, NSW - 1024], F32, kind="ExternalInput").ap()
    minv_d = nc.dram_tensor("minv", [128, 4 * BD], F32, kind="ExternalOutput").ap()
    minr_d = nc.dram_tensor("minr", [1, NSW], F32, kind="ExternalOutput").ap()

    with tile.TileContext(nc) as tc:
        with (
            tc.tile_pool(name="sb", bufs=1) as sb,
            tc.tile_pool(name="cp", bufs=2) as cpp,
            tc.tile_pool(name="pp", bufs=2, space=bass.MemorySpace.PSUM) as pp,
            tc.tile_pool(name="pw", bufs=4, space=bass.MemorySpace.PSUM) as pw,
        ):
            xta = sb.tile([20, 128 * GA], MM_DT)
            xtb = sb.tile([20, 128 * (BD - GA)], MM_DT)
            p20 = sb.tile([20, 512], MM_DT)
            auxa = sb.tile([5, 1152], MM_DT)
            xsb = sb.tile([5, NSW - 1024], MM_DT)
            minv = sb.tile([128, 4 * BD], F32)
            allres = sb.tile([128, NSW], F32)

            nc.gpsimd.dma_start(auxa[:], auxa_d.bitcast(MM_DT))
            nc.scalar.dma_start(p20[:], p20_d.bitcast(MM_DT))
            nc.sync.dma_start(xta[:], xta_d.bitcast(MM_DT))
            nc.sync.dma_start(xsb[:], xsb_d.bitcast(MM_DT))
            nc.sync.dma_start(xtb[:], xtb_d.bitcast(MM_DT))
            pal5 = auxa[:, 0:128]

            def bd_pair(p):
                src = xta if p < 6 else xtb
                g0 = 2 * p if p < 6 else 2 * (p - 6)  # local group index
                d_ps = pp.tile([128, 1024], F32)
                for h in range(2):
                    nc.tensor.matmul(
                        d_ps[:, 512 * h : 512 * (h + 1)],
                        src[:, 128 * (g0 + h) : 128 * (g0 + h + 1)],
                        p20[:],
                        start=True,
                        stop=True,
                    )
                base = 8 * p if p < 6 else 4 * GA + 8 * (p - 6)
                nc.vector.tensor_reduce(
                    minv[:, base : base + 8],
                    d_ps[:].rearrange("p (c j) -> p c j", j=128),
                    axis=mybir.AxisListType.X,
                    op=AluOpType.min,
                )

            def bd_single():
                g0 = GA - 1  # last group of xtA
                d_ps = pw.tile([128, 512], F32)
                nc.tensor.matmul(
                    d_ps[:],
                    xta[:, 128 * g0 : 128 * (g0 + 1)],
                    p20[:],
                    start=True,
                    stop=True,
                )
                nc.vector.tensor_reduce(
                    minv[:, 4 * g0 : 4 * g0 + 4],
                    d_ps[:].rearrange("p (c j) -> p c j", j=128),
                    axis=mybir.AxisListType.X,
                    op=AluOpType.min,
                )

            def sw_one(s):
                mov = auxa[:, 128 + 512 * s : 640 + 512 * s] if s < 2 else xsb[
                    :, 512 * (s - 2) : 512 * (s - 1)
                ]
                d_ps = pw.tile([128, 512], F32)
                nc.tensor.matmul(d_ps[:], pal5[:], mov, start=True, stop=True)
                cp = cpp.tile([128, 512], F32)
                nc.scalar.mul(cp[:], d_ps[:], -1.0)
                nc.gpsimd.partition_all_reduce(
                    allres[:, bass.ts(s, 512)],
                    cp[:],
                    channels=128,
                    reduce_op=bass_isa.ReduceOp.max,
                )

            for k in range(6):
                sw_one(k)
                bd_pair(k)
            sw_one(6)
            for p in range(6, 12):
                bd_pair(p)
            bd_single()

            nc.sync.dma_start(minv_d[:], minv[:])
            nc.sync.dma_start(minr_d[:], allres[0:1, :])

    nc.compile()
    return nc


def prep_inputs(output_colors, target_palette):
    pal = np.asarray(target_palette, dtype=np.float32)
    mu = pal.mean(axis=0)
    pp = pal - mu  # (128, 3) centered palette
    pn = (pp * pp).sum(axis=1)  # (128,)
    p20 = np.zeros((20, 512), dtype=np.float32)
    for c in range(4):
        p20[5 * c : 5 * c + 3, 128 * c : 128 * (c + 1)] = -2.0 * pp.T
        p20[5 * c + 3, 128 * c : 128 * (c + 1)] = pn
        p20[5 * c + 4, 128 * c : 128 * (c + 1)] = 1.0

    x = np.asarray(output_colors, dtype=np.float32) - mu
    in_maps = []
    for k in range(N_CORES):
        xs = x[k * NPC : (k + 1) * NPC]  # (16384, 3)
        xn2 = (xs * xs).sum(axis=1)  # (16384,)

        xb = xs[:NBD].reshape(BD, 4, 128, 3)  # [g, c, i, ch]
        nb = xn2[:NBD].reshape(BD, 4, 128)
        xt = np.empty((4, 5, BD, 128), dtype=np.float32)  # [c, row, g, i]
        xt[:, 0:3] = xb.transpose(1, 3, 0, 2)
        xt[:, 3] = 1.0
        xt[:, 4] = nb.transpose(1, 0, 2)
        xt = xt.reshape(20, BD * 128)

        xsw = np.empty((5, NSW), dtype=np.float32)
        xsw[0:3] = xs[NBD:].T
        xsw[3] = 1.0
        xsw[4] = xn2[NBD:]
        auxa = np.empty((5, 1152), dtype=np.float32)
        auxa[0:3, 0:128] = -2.0 * pp.T
        auxa[3, 0:128] = pn
        auxa[4, 0:128] = 1.0
        auxa[:, 128:1152] = xsw[:, 0:1024]

        in_maps.append(
            {
                "xta": np.ascontiguousarray(xt[:, : 128 * GA]),
                "xtb": np.ascontiguousarray(xt[:, 128 * GA :]),
                "p20": p20,
                "auxa": auxa,
                "xsb": np.ascontiguousarray(xsw[:, 1024:]),
            }
        )
    return in_maps


_NC_CACHE = {}


def get_nc():
    if "nc" not in _NC_CACHE:
        _NC_CACHE["nc"] = build_nc()
    return _NC_CACHE["nc"]


def kernel(output_colors=None, target_palette=None, _trace=False, **_):
    nc = get_nc()
    in_maps = prep_inputs(output_colors, target_palette)
    res = run_bass_kernel_spmd(
        nc, in_maps, core_ids=list(range(N_CORES)), trace=_trace
    )
    total = np.float64(0.0)
    for r in res.results:
        d2b = np.maximum(r["minv"].astype(np.float64), 0.0)
        d2s = np.maximum(-r["minr"].astype(np.float64), 0.0)
        total += np.sqrt(d2b).sum() + np.sqrt(d2s).sum()
    out = np.array(total / N, dtype=np.float32)
    if _trace:
        kernel._last_results = res
    return out


if __name__ == "__main__":
    rng = np.random.default_rng(0)
    oc = rng.random((N, 3), dtype=np.float32)
    tp = rng.random((M, 3), dtype=np.float32)
    got = kernel(output_colors=oc, target_palette=tp)
    d = oc[:, None, :] - tp[None, :, :]
    want = np.sqrt((d * d).sum(-1)).min(1).mean(dtype=np.float64)
    print("got", got, "want", want, "rel", abs(got - want) / abs(want))


# revision 21
# speedup vs baseline: 1.5411x; 1.0362x over previous
"""Nearest-color-distance loss on 8 TRN2 NeuronCores.

loss = mean_i min_j ||x_i - p_j||_2,  x: (131072, 3), p: (128, 3).

Per core (16384 colors): d2(i,j) = ||p_j||^2 - 2 x_i.p_j + ||x_i||^2
computed entirely inside the PE via 5-row packings (x_ch, 1, ||x||^2
against -2p_ch, ||p||^2, 1). Two layouts run interleaved so no single
reduction engine gates the loop:
 - 25 "bd" groups: 4 color-chunks block-diagonal (K=20) per matmul,
   colors on PSUM partitions; DVE min-reduces pairs of groups over
   the palette (free) axis (12 pairs + 1 single).
 - 7 "sw" groups: palette stationary (K=5), colors moving; palette on
   PSUM partitions; ACT copies PSUM->SBUF and GpSimd min-reduces over
   the partition (C) axis.
sw units are emitted first (lowest scheduler priority) so the slow
GpSimd chain starts as early as possible; inputs arrive in two
consolidated DMAs ([20 , *] matmul operands, [5, *] swapped operands).
Raw min-d2 go back to the host, which does sqrt/clamp/mean in f64,
plus layout + centering prep.
"""

import sys

sys.path.insert(0, "/opt/trn_rl_repo")

import numpy as np

import concourse.bass as bass
import concourse.bass_isa as bass_isa
import concourse.tile as tile
from concourse import bacc, mybir
from concourse.alu_op_type import AluOpType
from concourse.bass_utils import run_bass_kernel_spmd

N_CORES = 8
N = 131072
NPC = N // N_CORES  # 16384 colors per core
M = 128  # palette size
BD = 25  # block-diagonal groups of 512 colors (12 pairs + 1 single)
SW = 7  # swapped-layout groups (ACT+GpSimd-consumed)
NBD = BD * 512  # 12800 colors via bd path
NSW = NPC - NBD  # 3584 colors via sw path
WB = 128 * BD  # xt columns
F32 = mybir.dt.float32
F32R = mybir.dt.float32r
AF = mybir.ActivationFunctionType

MM_DT = F32R  # full-rate PE dtype; flip to F32 if precision fails


def build_nc():
    nc = bacc.Bacc(
        "TRN2",
        target_bir_lowering=False,
        debug=False,
        enable_asserts=False,
        num_devices=N_CORES,
    )
    big_d = nc.dram_tensor("big", [20, WB + 512], F32, kind="ExternalInput").ap()
    aux_d = nc.dram_tensor("aux", [5, 128 + NSW], F32, kind="ExternalInput").ap()
    minv_d = nc.dram_tensor("minv", [128, 4 * BD], F32, kind="ExternalOutput").ap()
    minr_d = nc.dram_tensor("minr", [1, NSW], F32, kind="ExternalOutput").ap()

    with tile.TileContext(nc) as tc:
        with (
            tc.tile_pool(name="sb", bufs=1) as sb,
            tc.tile_pool(name="cp", bufs=4) as cpp,
            tc.tile_pool(name="pp", bufs=3, space=bass.MemorySpace.PSUM) as pp,
            tc.tile_pool(name="pw", bufs=2, space=bass.MemorySpace.PSUM) as pw,
        ):
            big = sb.tile([20, WB + 512], MM_DT)
            aux = sb.tile([5, 128 + NSW], MM_DT)
            minv = sb.tile([128, 4 * BD], F32)
            allres = sb.tile([128, NSW], F32)

            nc.gpsimd.dma_start(aux[:], aux_d.bitcast(MM_DT))
            nc.sync.dma_start(big[:], big_d.bitcast(MM_DT))
            pal5 = aux[:, 0:128]
            p20 = big[:, WB : WB + 512]

            def sw_one(s):
                mov = aux[:, 128 + 512 * s : 640 + 512 * s]
                d_ps = pw.tile([128, 512], F32)
                nc.tensor.matmul(d_ps[:], pal5[:], mov, start=True, stop=True)
                cp = cpp.tile([128, 512], F32)
                nc.scalar.mul(cp[:], d_ps[:], -1.0)
                nc.gpsimd.partition_all_reduce(
                    allres[:, bass.ts(s, 512)],
                    cp[:],
                    channels=128,
                    reduce_op=bass_isa.ReduceOp.max,
                )

            def bd_pair(p):
                d_ps = pp.tile([128, 1024], F32)
                for h in range(2):
                    nc.tensor.matmul(
                        d_ps[:, 512 * h : 512 * (h + 1)],
                        big[:, 128 * (2 * p + h) : 128 * (2 * p + h + 1)],
                        p20,
                        start=True,
                        stop=True,
                    )
                nc.vector.tensor_reduce(
                    minv[:, 8 * p : 8 * p + 8],
                    d_ps[:].rearrange("p (c j) -> p c j", j=128),
                    axis=mybir.AxisListType.X,
                    op=AluOpType.min,
                )

            def bd_single():
                d_ps = pp.tile([128, 1024], F32)
                nc.tensor.matmul(
                    d_ps[:, 0:512],
                    big[:, 128 * (BD - 1) : 128 * BD],
                    p20,
                    start=True,
                    stop=True,
                )
                nc.vector.tensor_reduce(
                    minv[:, 4 * (BD - 1) : 4 * BD],
                    d_ps[:, 0:512].rearrange("p (c j) -> p c j", j=128),
                    axis=mybir.AxisListType.X,
                    op=AluOpType.min,
                )

            for s in range(SW):
                sw_one(s)
            for p in range(12):
                bd_pair(p)
            bd_single()

            nc.sync.dma_start(minr_d[:], allres[0:1, :])
            nc.sync.dma_start(minv_d[:], minv[:])

    nc.compile()
    return nc


def prep_inputs(output_colors, target_palette):
    pal = np.asarray(target_palette, dtype=np.float32)
    mu = pal.mean(axis=0)
    pp = pal - mu  # (128, 3) centered palette
    pn = (pp * pp).sum(axis=1)  # (128,)

    x = np.asarray(output_colors, dtype=np.float32) - mu
    in_maps = []
    for k in range(N_CORES):
        xs = x[k * NPC : (k + 1) * NPC]  # (16384, 3)
        xn2 = (xs * xs).sum(axis=1)  # (16384,)

        xb = xs[:NBD].reshape(BD, 4, 128, 3)  # [g, c, i, ch]
        nb = xn2[:NBD].reshape(BD, 4, 128)
        xt = np.empty((4, 5, BD, 128), dtype=np.float32)  # [c, row, g, i]
        xt[:, 0:3] = xb.transpose(1, 3, 0, 2)
        xt[:, 3] = 1.0
        xt[:, 4] = nb.transpose(1, 0, 2)
        p20 = np.zeros((20, 512), dtype=np.float32)
        for c in range(4):
            p20[5 * c : 5 * c + 3, 128 * c : 128 * (c + 1)] = -2.0 * pp.T
            p20[5 * c + 3, 128 * c : 128 * (c + 1)] = pn
            p20[5 * c + 4, 128 * c : 128 * (c + 1)] = 1.0
        bigf = np.empty((20, WB + 512), dtype=np.float32)
        bigf[:, :WB] = xt.reshape(20, WB)
        bigf[:, WB:] = p20

        aux = np.empty((5, 128 + NSW), dtype=np.float32)
        aux[0:3, 0:128] = -2.0 * pp.T
        aux[3, 0:128] = pn
        aux[4, 0:128] = 1.0
        aux[0:3, 128:] = xs[NBD:].T
        aux[3, 128:] = 1.0
        aux[4, 128:] = xn2[NBD:]

        in_maps.append({"big": bigf, "aux": aux})
    return in_maps


_NC_CACHE = {}


def get_nc():
    if "nc" not in _NC_CACHE:
        _NC_CACHE["nc"] = build_nc()
    return _NC_CACHE["nc"]


def kernel(output_colors=None, target_palette=None, _trace=False, **_):
    nc = get_nc()
    in_maps = prep_inputs(output_colors, target_palette)
    res = run_bass_kernel_spmd(
        nc, in_maps, core_ids=list(range(N_CORES)), trace=_trace
    )
    total = np.float64(0.0)
    for r in res.results:
        d2b = np.maximum(r["minv"].astype(np.float64), 0.0)
        d2s = np.maximum(-r["minr"].astype(np.float64), 0.0)
        total += np.sqrt(d2b).sum() + np.sqrt(d2s).sum()
    out = np.array(total / N, dtype=np.float32)
    if _trace:
        kernel._last_results = res
    return out


if __name__ == "__main__":
    rng = np.random.default_rng(0)
    oc = rng.random((N, 3), dtype=np.float32)
    tp = rng.random((M, 3), dtype=np.float32)
    got = kernel(output_colors=oc, target_palette=tp)
    d = oc[:, None, :] - tp[None, :, :]
    want = np.sqrt((d * d).sum(-1)).min(1).mean(dtype=np.float64)
    print("got", got, "want", want, "rel", abs(got - want) / abs(want))


# revision 26
# speedup vs baseline: 1.5596x; 1.0120x over previous
"""Nearest-color-distance loss on 8 TRN2 NeuronCores.

loss = mean_i min_j ||x_i - p_j||_2,  x: (131072, 3), p: (128, 3).

Per core (16384 colors): d2(i,j) = ||p_j||^2 - 2 x_i.p_j + ||x_i||^2
computed entirely inside the PE via 5-row packings (x_ch, 1, ||x||^2
against -2p_ch, ||p||^2, 1). Two layouts run interleaved so no single
reduction engine gates the loop:
 - 25 "bd" groups: 4 color-chunks block-diagonal (K=20) per matmul,
   colors on PSUM partitions; DVE min-reduces pairs of groups over
   the palette (free) axis (12 pairs + 1 single).
 - 7 "sw" groups: palette stationary (K=5), colors moving; palette on
   PSUM partitions; ACT copies PSUM->SBUF and GpSimd min-reduces over
   the partition (C) axis.
sw units are emitted first (lowest scheduler priority) so the slow
GpSimd chain starts as early as possible; inputs arrive in two
consolidated DMAs ([20 , *] matmul operands, [5, *] swapped operands).
Raw min-d2 go back to the host, which does sqrt/clamp/mean in f64,
plus layout + centering prep.
"""

import sys

sys.path.insert(0, "/opt/trn_rl_repo")

import numpy as np

import concourse.bass as bass
import concourse.bass_isa as bass_isa
import concourse.tile as tile
from concourse import bacc, mybir
from concourse.alu_op_type import AluOpType
from concourse.bass_utils import run_bass_kernel_spmd

N_CORES = 8
N = 131072
NPC = N // N_CORES  # 16384 colors per core
M = 128  # palette size
BD = 25  # block-diagonal groups of 512 colors (12 pairs + 1 single)
SW = 7  # swapped-layout groups (ACT+GpSimd-consumed)
NBD = BD * 512  # 12800 colors via bd path
NSW = NPC - NBD  # 3584 colors via sw path
WB = 128 * BD  # xt columns
F32 = mybir.dt.float32
F32R = mybir.dt.float32r
AF = mybir.ActivationFunctionType

MM_DT = F32R  # full-rate PE dtype; flip to F32 if precision fails


def build_nc():
    nc = bacc.Bacc(
        "TRN2",
        target_bir_lowering=False,
        debug=False,
        enable_asserts=False,
        num_devices=N_CORES,
    )
    aux1_d = nc.dram_tensor("aux1", [5, 1152], F32, kind="ExternalInput").ap()
    aux2_d = nc.dram_tensor("aux2", [5, NSW - 1024], F32, kind="ExternalInput").ap()
    p20_d = nc.dram_tensor("p20", [20, 512], F32, kind="ExternalInput").ap()
    xt1_d = nc.dram_tensor("xt1", [20, 1280], F32, kind="ExternalInput").ap()
    xt2_d = nc.dram_tensor("xt2", [20, WB - 1280], F32, kind="ExternalInput").ap()
    minv_d = nc.dram_tensor("minv", [128, 4 * BD], F32, kind="ExternalOutput").ap()
    minr_d = nc.dram_tensor("minr", [1, NSW], F32, kind="ExternalOutput").ap()

    with tile.TileContext(nc) as tc:
        with (
            tc.tile_pool(name="sb", bufs=1) as sb,
            tc.tile_pool(name="cp", bufs=4) as cpp,
            tc.tile_pool(name="pp", bufs=3, space=bass.MemorySpace.PSUM) as pp,
            tc.tile_pool(name="pw", bufs=2, space=bass.MemorySpace.PSUM) as pw,
        ):
            aux1 = sb.tile([5, 1152], MM_DT)
            aux2 = sb.tile([5, NSW - 1024], MM_DT)
            p20t = sb.tile([20, 512], MM_DT)
            xt1 = sb.tile([20, 1280], MM_DT)
            xt2 = sb.tile([20, WB - 1280], MM_DT)
            minv = sb.tile([128, 4 * BD], F32)
            allres = sb.tile([128, NSW], F32)

            nc.scalar.dma_start(aux1[:], aux1_d.bitcast(MM_DT))
            nc.scalar.dma_start(p20t[:], p20_d.bitcast(MM_DT))
            nc.sync.dma_start(xt1[:], xt1_d.bitcast(MM_DT))
            nc.sync.dma_start(aux2[:], aux2_d.bitcast(MM_DT))
            nc.sync.dma_start(xt2[:], xt2_d.bitcast(MM_DT))
            pal5 = aux1[:, 0:128]
            p20 = p20t[:]

            def sw_one(s):
                mov = (
                    aux1[:, 128 + 512 * s : 640 + 512 * s]
                    if s < 2
                    else aux2[:, 512 * (s - 2) : 512 * (s - 1)]
                )
                d_ps = pw.tile([128, 512], F32)
                nc.tensor.matmul(d_ps[:], pal5[:], mov, start=True, stop=True)
                cp = cpp.tile([128, 512], F32)
                nc.scalar.mul(cp[:], d_ps[:], -1.0)
                nc.gpsimd.partition_all_reduce(
                    allres[:, bass.ts(s, 512)],
                    cp[:],
                    channels=128,
                    reduce_op=bass_isa.ReduceOp.max,
                )

            def bd_pair(p):
                d_ps = pp.tile([128, 1024], F32)
                for h in range(2):
                    g = 2 * p + h
                    src = xt1[:, 128 * g : 128 * (g + 1)] if g < 10 else xt2[
                        :, 128 * (g - 10) : 128 * (g - 9)
                    ]
                    nc.tensor.matmul(
                        d_ps[:, 512 * h : 512 * (h + 1)],
                        src,
                        p20,
                        start=True,
                        stop=True,
                    )
                nc.vector.tensor_reduce(
                    minv[:, 8 * p : 8 * p + 8],
                    d_ps[:].rearrange("p (c j) -> p c j", j=128),
                    axis=mybir.AxisListType.X,
                    op=AluOpType.min,
                )

            def bd_single():
                d_ps = pp.tile([128, 1024], F32)
                nc.tensor.matmul(
                    d_ps[:, 0:512],
                    xt2[:, 128 * (BD - 11) : 128 * (BD - 10)],
                    p20,
                    start=True,
                    stop=True,
                )
                nc.vector.tensor_reduce(
                    minv[:, 4 * (BD - 1) : 4 * BD],
                    d_ps[:, 0:512].rearrange("p (c j) -> p c j", j=128),
                    axis=mybir.AxisListType.X,
                    op=AluOpType.min,
                )

            for s in range(SW):
                sw_one(s)
            for p in range(12):
                bd_pair(p)
            bd_single()

            nc.sync.dma_start(minr_d[:], allres[0:1, :])
            nc.sync.dma_start(minv_d[:], minv[:])

    nc.compile()
    return nc


def prep_inputs(output_colors, target_palette):
    pal = np.asarray(target_palette, dtype=np.float32)
    mu = pal.mean(axis=0)
    pp = pal - mu  # (128, 3) centered palette
    pn = (pp * pp).sum(axis=1)  # (128,)

    x = np.asarray(output_colors, dtype=np.float32) - mu
    in_maps = []
    for k in range(N_CORES):
        xs = x[k * NPC : (k + 1) * NPC]  # (16384, 3)
        xn2 = (xs * xs).sum(axis=1)  # (16384,)

        xb = xs[:NBD].reshape(BD, 4, 128, 3)  # [g, c, i, ch]
        nb = xn2[:NBD].reshape(BD, 4, 128)
        xt = np.empty((4, 5, BD, 128), dtype=np.float32)  # [c, row, g, i]
        xt[:, 0:3] = xb.transpose(1, 3, 0, 2)
        xt[:, 3] = 1.0
        xt[:, 4] = nb.transpose(1, 0, 2)
        xt = xt.reshape(20, WB)
        p20 = np.zeros((20, 512), dtype=np.float32)
        for c in range(4):
            p20[5 * c : 5 * c + 3, 128 * c : 128 * (c + 1)] = -2.0 * pp.T
            p20[5 * c + 3, 128 * c : 128 * (c + 1)] = pn
            p20[5 * c + 4, 128 * c : 128 * (c + 1)] = 1.0

        xsw = np.empty((5, NSW), dtype=np.float32)
        xsw[0:3] = xs[NBD:].T
        xsw[3] = 1.0
        xsw[4] = xn2[NBD:]
        aux1 = np.empty((5, 1152), dtype=np.float32)
        aux1[0:3, 0:128] = -2.0 * pp.T
        aux1[3, 0:128] = pn
        aux1[4, 0:128] = 1.0
        aux1[:, 128:] = xsw[:, 0:1024]

        in_maps.append(
            {
                "aux1": aux1,
                "aux2": np.ascontiguousarray(xsw[:, 1024:]),
                "p20": p20,
                "xt1": np.ascontiguousarray(xt[:, :1280]),
                "xt2": np.ascontiguousarray(xt[:, 1280:]),
            }
        )
    return in_maps


_NC_CACHE = {}


def get_nc():
    if "nc" not in _NC_CACHE:
        _NC_CACHE["nc"] = build_nc()
    return _NC_CACHE["nc"]


def kernel(output_colors=None, target_palette=None, _trace=False, **_):
    nc = get_nc()
    in_maps = prep_inputs(output_colors, target_palette)
    res = run_bass_kernel_spmd(
        nc, in_maps, core_ids=list(range(N_CORES)), trace=_trace
    )
    total = np.float64(0.0)
    for r in res.results:
        d2b = np.maximum(r["minv"].astype(np.float64), 0.0)
        d2s = np.maximum(-r["minr"].astype(np.float64), 0.0)
        total += np.sqrt(d2b).sum() + np.sqrt(d2s).sum()
    out = np.array(total / N, dtype=np.float32)
    if _trace:
        kernel._last_results = res
    return out


if __name__ == "__main__":
    rng = np.random.default_rng(0)
    oc = rng.random((N, 3), dtype=np.float32)
    tp = rng.random((M, 3), dtype=np.float32)
    got = kernel(output_colors=oc, target_palette=tp)
    d = oc[:, None, :] - tp[None, :, :]
    want = np.sqrt((d * d).sum(-1)).min(1).mean(dtype=np.float64)
    print("got", got, "want", want, "rel", abs(got - want) / abs(want))


# revision 28
# speedup vs baseline: 1.6587x; 1.0636x over previous
"""Nearest-color-distance loss on 8 TRN2 NeuronCores.

loss = mean_i min_j ||x_i - p_j||_2,  x: (131072, 3), p: (128, 3).

Per core (16384 colors): d2(i,j) = ||p_j||^2 - 2 x_i.p_j + ||x_i||^2
computed entirely inside the PE via 5-row packings (x_ch, 1, ||x||^2
against -2p_ch, ||p||^2, 1). Two layouts run interleaved so no single
reduction engine gates the loop:
 - 27 "bd" groups: 4 color-chunks block-diagonal (K=20) per matmul,
   colors on PSUM partitions; DVE min-reduces pairs of groups over the
   palette (free) axis (13 pairs + 1 single).
 - 5 "sw" groups: palette stationary (K=5), colors moving; palette on
   PSUM partitions; ACT negate-copies PSUM->SBUF and GpSimd max-reduces
   over the partition (C) axis (no min op -> negate trick).
The gpsimd PartitionAllReduce library load takes ~7.6us in the
background, so no DMA is placed on the gpsimd queue (LIBRARY_RELOAD
issues right after pool init). p20/xt1 are staged first and small so
the bd pipeline starts ASAP; outputs are split so result DMAs overlap
the tails of the reduce chains. Raw min-d2 go back to the host, which
does sqrt/clamp/mean in f64, plus layout + centering prep.
"""

import sys

sys.path.insert(0, "/opt/trn_rl_repo")

import numpy as np

import concourse.bass as bass
import concourse.bass_isa as bass_isa
import concourse.tile as tile
from concourse import bacc, mybir
from concourse.alu_op_type import AluOpType
from concourse.bass_utils import run_bass_kernel_spmd

N_CORES = 8
N = 131072
NPC = N // N_CORES  # 16384 colors per core
M = 128  # palette size
BD = 27  # block-diagonal groups of 512 colors (13 pairs + 1 single)
SW = 5  # swapped-layout groups (ACT+GpSimd-consumed)
NBD = BD * 512  # 13824 colors via bd path
NSW = NPC - NBD  # 2560 colors via sw path
WB = 128 * BD  # 3456 xt columns
F32 = mybir.dt.float32
F32R = mybir.dt.float32r
AF = mybir.ActivationFunctionType

MM_DT = F32R  # full-rate PE dtype; flip to F32 if precision fails


def build_nc():
    nc = bacc.Bacc(
        "TRN2",
        target_bir_lowering=False,
        debug=False,
        enable_asserts=False,
        num_devices=N_CORES,
    )
    aux1_d = nc.dram_tensor("aux1", [5, 1152], F32, kind="ExternalInput").ap()
    aux2_d = nc.dram_tensor("aux2", [5, NSW - 1024], F32, kind="ExternalInput").ap()
    p20_d = nc.dram_tensor("p20", [20, 512], F32, kind="ExternalInput").ap()
    xt1_d = nc.dram_tensor("xt1", [20, 512], F32, kind="ExternalInput").ap()
    xt2_d = nc.dram_tensor("xt2", [20, WB - 512], F32, kind="ExternalInput").ap()
    minva_d = nc.dram_tensor("minva", [128, 88], F32, kind="ExternalOutput").ap()
    minvb_d = nc.dram_tensor("minvb", [128, 20], F32, kind="ExternalOutput").ap()
    minr1_d = nc.dram_tensor("minr1", [1, 1536], F32, kind="ExternalOutput").ap()
    minr2_d = nc.dram_tensor("minr2", [1, 1024], F32, kind="ExternalOutput").ap()

    with tile.TileContext(nc) as tc:
        with (
            tc.tile_pool(name="sb", bufs=1) as sb,
            tc.tile_pool(name="cp", bufs=4) as cpp,
            tc.tile_pool(name="pp", bufs=3, space=bass.MemorySpace.PSUM) as pp,
            tc.tile_pool(name="pw", bufs=2, space=bass.MemorySpace.PSUM) as pw,
        ):
            aux1 = sb.tile([5, 1152], MM_DT)
            aux2 = sb.tile([5, NSW - 1024], MM_DT)
            p20t = sb.tile([20, 512], MM_DT)
            xt1 = sb.tile([20, 512], MM_DT)
            xt2 = sb.tile([20, WB - 512], MM_DT)
            minva = sb.tile([128, 88], F32)
            minvb = sb.tile([128, 20], F32)
            allra = sb.tile([128, 1536], F32)
            allrb = sb.tile([128, 1024], F32)

            nc.scalar.dma_start(p20t[:], p20_d.bitcast(MM_DT))
            nc.scalar.dma_start(aux1[:], aux1_d.bitcast(MM_DT))
            nc.sync.dma_start(xt1[:], xt1_d.bitcast(MM_DT))
            nc.sync.dma_start(xt2[:], xt2_d.bitcast(MM_DT))
            nc.sync.dma_start(aux2[:], aux2_d.bitcast(MM_DT))
            pal5 = aux1[:, 0:128]
            p20 = p20t[:]

            def sw_one(s):
                mov = (
                    aux1[:, 128 + 512 * s : 640 + 512 * s]
                    if s < 2
                    else aux2[:, 512 * (s - 2) : 512 * (s - 1)]
                )
                d_ps = pw.tile([128, 512], F32)
                nc.tensor.matmul(d_ps[:], pal5[:], mov, start=True, stop=True)
                cp = cpp.tile([128, 512], F32)
                nc.scalar.mul(cp[:], d_ps[:], -1.0)
                dst = (
                    allra[:, bass.ts(s, 512)]
                    if s < 3
                    else allrb[:, bass.ts(s - 3, 512)]
                )
                nc.gpsimd.partition_all_reduce(
                    dst,
                    cp[:],
                    channels=128,
                    reduce_op=bass_isa.ReduceOp.max,
                )

            def bd_pair(p):
                d_ps = pp.tile([128, 1024], F32)
                for h in range(2):
                    g = 2 * p + h
                    src = xt1[:, 128 * g : 128 * (g + 1)] if g < 4 else xt2[
                        :, 128 * (g - 4) : 128 * (g - 3)
                    ]
                    nc.tensor.matmul(
                        d_ps[:, 512 * h : 512 * (h + 1)],
                        src,
                        p20,
                        start=True,
                        stop=True,
                    )
                out = (
                    minva[:, 8 * p : 8 * p + 8]
                    if p < 11
                    else minvb[:, 8 * (p - 11) : 8 * (p - 11) + 8]
                )
                nc.vector.tensor_reduce(
                    out,
                    d_ps[:].rearrange("p (c j) -> p c j", j=128),
                    axis=mybir.AxisListType.X,
                    op=AluOpType.min,
                )

            def bd_single():
                d_ps = pp.tile([128, 1024], F32)
                nc.tensor.matmul(
                    d_ps[:, 0:512],
                    xt2[:, 128 * (BD - 5) : 128 * (BD - 4)],
                    p20,
                    start=True,
                    stop=True,
                )
                nc.vector.tensor_reduce(
                    minvb[:, 16:20],
                    d_ps[:, 0:512].rearrange("p (c j) -> p c j", j=128),
                    axis=mybir.AxisListType.X,
                    op=AluOpType.min,
                )

            sw_one(0)
            sw_one(1)
            bd_pair(0)
            bd_pair(1)
            sw_one(2)
            bd_pair(2)
            sw_one(3)
            bd_pair(3)
            sw_one(4)
            for p in range(4, 13):
                bd_pair(p)
            bd_single()

            nc.scalar.dma_start(minr1_d[:], allra[0:1, :])
            nc.scalar.dma_start(minr2_d[:], allrb[0:1, :])
            nc.sync.dma_start(minva_d[:], minva[:])
            nc.sync.dma_start(minvb_d[:], minvb[:])

    nc.compile()
    return nc


def prep_inputs(output_colors, target_palette):
    pal = np.asarray(target_palette, dtype=np.float32)
    mu = pal.mean(axis=0)
    pp = pal - mu  # (128, 3) centered palette
    pn = (pp * pp).sum(axis=1)  # (128,)

    p20 = np.zeros((20, 512), dtype=np.float32)
    for c in range(4):
        p20[5 * c : 5 * c + 3, 128 * c : 128 * (c + 1)] = -2.0 * pp.T
        p20[5 * c + 3, 128 * c : 128 * (c + 1)] = pn
        p20[5 * c + 4, 128 * c : 128 * (c + 1)] = 1.0

    x = np.asarray(output_colors, dtype=np.float32) - mu
    in_maps = []
    for k in range(N_CORES):
        xs = x[k * NPC : (k + 1) * NPC]  # (16384, 3)
        xn2 = (xs * xs).sum(axis=1)  # (16384,)

        xb = xs[:NBD].reshape(BD, 4, 128, 3)  # [g, c, i, ch]
        nb = xn2[:NBD].reshape(BD, 4, 128)
        xt = np.empty((4, 5, BD, 128), dtype=np.float32)  # [c, row, g, i]
        xt[:, 0:3] = xb.transpose(1, 3, 0, 2)
        xt[:, 3] = 1.0
        xt[:, 4] = nb.transpose(1, 0, 2)
        xt = xt.reshape(20, WB)

        xsw = np.empty((5, NSW), dtype=np.float32)
        xsw[0:3] = xs[NBD:].T
        xsw[3] = 1.0
        xsw[4] = xn2[NBD:]
        aux1 = np.empty((5, 1152), dtype=np.float32)
        aux1[0:3, 0:128] = -2.0 * pp.T
        aux1[3, 0:128] = pn
        aux1[4, 0:128] = 1.0
        aux1[:, 128:] = xsw[:, 0:1024]

        in_maps.append(
            {
                "aux1": aux1,
                "aux2": np.ascontiguousarray(xsw[:, 1024:]),
                "p20": p20,
                "xt1": np.ascontiguousarray(xt[:, :512]),
                "xt2": np.ascontiguousarray(xt[:, 512:]),
            }
        )
    return in_maps


_NC_CACHE = {}


def get_nc():
    if "nc" not in _NC_CACHE:
        _NC_CACHE["nc"] = build_nc()
    return _NC_CACHE["nc"]


def kernel(output_colors=None, target_palette=None, _trace=False, **_):
    nc = get_nc()
    in_maps = prep_inputs(output_colors, target_palette)
    res = run_bass_kernel_spmd(
        nc, in_maps, core_ids=list(range(N_CORES)), trace=_trace
    )
    total = np.float64(0.0)
    for r in res.results:
        mv = np.concatenate([r["minva"], r["minvb"]], axis=1)
        mr = np.concatenate([r["minr1"], r["minr2"]], axis=1)
        d2b = np.maximum(mv.astype(np.float64), 0.0)
        d2s = np.maximum(-mr.astype(np.float64), 0.0)
        total += np.sqrt(d2b).sum() + np.sqrt(d2s).sum()
    out = np.array(total / N, dtype=np.float32)
    if _trace:
        kernel._last_results = res
    return out


if __name__ == "__main__":
    rng = np.random.default_rng(0)
    oc = rng.random((N, 3), dtype=np.float32)
    tp = rng.random((M, 3), dtype=np.float32)
    got = kernel(output_colors=oc, target_palette=tp)
    d = oc[:, None, :] - tp[None, :, :]
    want = np.sqrt((d * d).sum(-1)).min(1).mean(dtype=np.float64)
    print("got", got, "want", want, "rel", abs(got - want) / abs(want))
